# revision 52
# baseline (speedup 1.0000x reference)
"""Contextual-attention kernel for Trainium2, batch-parallel over 8 NeuronCores.

Per core (one image, feature [256,64,64], shared mask [128,128]):
  1. fp2 = zero-bordered feature halves [128, 68, 68] (f at [1:65,1:65]);
     fdpj[cc][j] = contiguous [34,32] col-windows (j=0..2) of the padded
     downsampled feature (f32r) -> gram operands need no per-strip staging;
     plane[cc][(a,b,tw)] = bf16 parity-plane col-windows of fp2 (GpSimd
     copies) -> deconv patch-bank transposes get single-run lhsT views.
  2. Gram S[q,p]: 18 accumulating matmuls per [128,512] tile, lhsT and rhs
     both strided fdpj views; scaled by 1/||patch|| (squares + box sums +
     one channel-sum matmul per half, f32r fast path).
  3. fuse conv 1 (diag +-1 row-major) and 2 (diag +-1 col-major with wrap)
     as PE shift-matrix matmuls accumulating in PSUM. Score tiles carry
     2 zero guard cols each side so every big matmul is a full aligned
     512-wide write (has_written gives correct edge behavior); fuse2's
     p-wrap slivers are aligned 32-wide matmuls from staged scratches.
     One DVE add per chunk-half evacuates PSUM.
  4. softmax in p-major layout, fully per-128-col-block pipelined:
     PE-transpose masked scores -> DVE reduce_max -> per-partition bias ->
     ScalarE exp with accum_out summing the denominator in the same op ->
     reciprocal -> transpose back through diag(rcp) stationaries (scales
     each p column) -> evacuate with per-q mask mult into A_pad.
  5. RW deconv banks in fp8 hi/lo block pairs: per tap 8 bf16 transposes
     fill a PSUM bank; hi = e4m3 round (ScalarE), lo = residual (DVE).
     Only taps u in {0,1} are built; taps u+2 are 32-partition shifts,
     derived by two SBUF DMAs + a one-strip sliver. rw0's build is
     interleaved into the fuse phase, rw1's after the softmax so its
     evacs overlap deconv-cc0 PE time.
  6. A stored once as fp8 in u16 cells (zero high bytes): every deconv
     window, for any x-shift, is a stride-2 fp8 run at an even byte --
     a legal DoubleRow moving AP. Row-edge wrap artifacts of the +-1
     shifts are cancelled by N=16 correction matmuls vs negated edge
     columns, added onto the 16 edge pixels at evac.
  7. deconv: 512 accumulating fp8 DoubleRow matmuls (hi+lo slot pairs,
     0.5 cycles/row) -- half the bf16 PE cost at ~bf16 precision; accs
     complete serially so evacs/stores overlap compute.
"""
import sys

sys.path.insert(0, "/opt/trn_rl_repo")

import numpy as np

import concourse.bass as bass
import concourse.bacc as bacc_mod
import concourse.mybir as mybir
import concourse.tile as tile
from concourse.masks import make_identity
from concourse.bass_utils import run_bass_kernel_spmd

F32 = mybir.dt.float32
F32R = mybir.dt.float32r
BF16 = mybir.dt.bfloat16
F8 = mybir.dt.float8e4
U16 = mybir.dt.uint16
AX = mybir.AxisListType
OP = mybir.AluOpType
ACT = mybir.ActivationFunctionType
DROW = mybir.MatmulPerfMode.DoubleRow

N_CORES = 8
C, H, W = 256, 64, 64
SCALE = 10.0
EPS = 1e-4
G = 2  # guard columns each side of the 1024-wide score rows


def build_nc(gram_dt="f32r", dec_dt="bf16", reps=1, dbg=False):
    nc = bacc_mod.Bacc("TRN2", target_bir_lowering=False, debug=False)
    feat = nc.dram_tensor("feature", [C, H, W], F32, kind="ExternalInput")
    mask0 = nc.dram_tensor("mask0", [128, 128], F32, kind="ExternalInput")
    out_d = nc.dram_tensor("out", [C, H, W], F32, kind="ExternalOutput")
    dbg_d = None
    if dbg:
        dbg_d = {
            "d_m0": nc.dram_tensor("d_m0", [128, 8, 1028], F32, kind="ExternalOutput"),
            "d_m1": nc.dram_tensor("d_m1", [128, 8, 1028], F32, kind="ExternalOutput"),
            "d_m2": nc.dram_tensor("d_m2", [128, 8, 1028], F32, kind="ExternalOutput"),
            "d_ap": nc.dram_tensor("d_ap", [128, 2, 8, 34, 32], F8, kind="ExternalOutput"),
            "d_rn": nc.dram_tensor("d_rn", [128, 8], F32, kind="ExternalOutput"),
            "d_mm": nc.dram_tensor("d_mm", [128, 8], F32, kind="ExternalOutput"),
            "d_et": nc.dram_tensor("d_et", [128, 8, 1024], F32, kind="ExternalOutput"),
            "d_fj": nc.dram_tensor("d_fj", [128, 6, 34, 32], F32, kind="ExternalOutput"),
        }

    with tile.TileContext(nc) as tc:
        with (
            tc.tile_pool(name="big", bufs=3) as big,      # 3 x 32.9 KB
            tc.tile_pool(name="fpa", bufs=2) as fpa,      # 2 x 18.5 KB
            tc.tile_pool(name="fdj", bufs=6) as fdj,      # 6 x 4.25 KB
            tc.tile_pool(name="pln", bufs=8) as pln,      # 8 x 2.125 KB
            tc.tile_pool(name="etp", bufs=3) as etp,      # 3 x 2 KB
            tc.tile_pool(name="shr", bufs=2) as shr,      # raw shift staging
            tc.tile_pool(name="sml", bufs=1) as sml,
            tc.tile_pool(name="psp", bufs=8, space="PSUM") as psp,
        ):
            for rep in range(reps):
                _body(nc, tc, big, fpa, fdj, pln, etp, shr, sml,
                      psp, feat, mask0, out_d, rep, dbg_d)
    nc.finalize()
    return nc


def _make_shift(nc, t, delta):
    """t[k, q] = 1 iff k - q == delta (so (t.T @ M)[q, :] = M[q + delta, :])."""
    nc.gpsimd.memset(t, 0.0)
    nc.gpsimd.affine_select(
        out=t, in_=t, compare_op=OP.not_equal, fill=1.0,
        base=-delta, pattern=[[-1, 128]], channel_multiplier=1)


def _keep_rows(nc, t, lo, hi):
    """Zero partitions outside [lo, hi]."""
    nc.gpsimd.affine_select(
        out=t, in_=t, compare_op=OP.is_ge, fill=0.0,
        base=-lo, pattern=[[0, 128]], channel_multiplier=1)
    nc.gpsimd.affine_select(
        out=t, in_=t, compare_op=OP.is_ge, fill=0.0,
        base=hi, pattern=[[0, 128]], channel_multiplier=-1)


def _emit_group(nc, terms):
    n = len(terms)
    for i, (o, l, r) in enumerate(terms):
        nc.tensor.matmul(o, l, r, start=(i == 0), stop=(i == n - 1),
                         skip_group_check=True)


def _body(nc, tc, big, fpa, fdj, pln, etp, shr, sml, psp,
          feat, mask0, out_d, rep, dbg_d=None):
    DW = 1024 + 2 * G  # guarded score row width

    # ---------------- constants ----------------
    identf = shr.tile([128, 128], F32, tag="shraw", name=f"identf_{rep}")
    make_identity(nc, identf)
    ident_r = sml.tile([128, 128], F32R, tag="ident_r")
    nc.vector.tensor_copy(ident_r[:], identf[:])
    ident_bf = sml.tile([128, 128], BF16, tag="ident_bf")
    nc.vector.tensor_copy(ident_bf[:], identf[:])

    onesf = shr.tile([128, 128], F32, tag="ones_hold", name=f"onesf_{rep}")
    nc.vector.memset(onesf[:], 1.0)
    ones_r = sml.tile([128, 128], F32R, tag="ones_r")
    nc.vector.tensor_copy(ones_r[:], onesf[:])
    zf = shr.tile([128, 128], F32, tag="shraw", name=f"zf_{rep}")
    nc.vector.memset(zf[:], 0.0)
    zer = sml.tile([128, 64], F32R, tag="zer")
    nc.vector.tensor_copy(zer[:], zf[:, 0:64])
    nc.scalar.sqrt(zf[0:1, 64:66], zf[0:1, 64:66])  # preload Sqrt act table

    shn = ("p1", "m1", "p32", "m32", "bp1", "bm1", "bp32", "bm32", "wp", "wm")
    shd = (1, -1, 32, -32, -127, 127, -96, 96, -95, 95)
    sh = {}
    for nm, d in zip(shn, shd):
        traw = shr.tile([128, 128], F32, tag="shraw", name=f"shr_{rep}_{nm}")
        _make_shift(nc, traw[:], d)
        if nm == "wp":
            _keep_rows(nc, traw[:], 1, 31)
        elif nm == "wm":
            _keep_rows(nc, traw[:], 96, 126)
        t = sml.tile([128, 128], F32R, tag=f"sh_{nm}", name=f"sh_{rep}_{nm}")
        nc.vector.tensor_copy(t[:], traw[:])
        sh[nm] = t
    slv = {}
    for nm in ("pm", "pb", "mm", "mb"):
        t = sml.tile([128, 32], F32R, tag=f"sl_{nm}", name=f"sl_{rep}_{nm}")
        nc.vector.tensor_copy(t[:], zer[:, 0:32])
        slv[nm] = t

    # ---------------- feature loads (first: contiguous, HWDGE-priority) ----
    # fp2[cc]: [128, 68, 64]; feature rows at 1..64 (full-width), row pads 0
    # and 65..67 zero. Column edges are handled by the staging copies.
    fp2 = []
    for cc in range(2):
        t = fpa.tile([128, 68, 64], F32, tag="fpa", name=f"fp2_{rep}_{cc}")
        nc.sync.dma_start(t[:, 1:33, :], feat[cc * 128:(cc + 1) * 128, 0:32])
        nc.sync.dma_start(t[:, 33:65, :], feat[cc * 128:(cc + 1) * 128, 32:64])
        nc.vector.memset(t[:, 0:1, :], 0.0)
        nc.vector.memset(t[:, 65:68, :], 0.0)
        fp2.append(t)

    # ---------------- shared scratch (mask then norm) ----------------
    scr = big.tile([128, 2180], F32, tag="big", name=f"scr_{rep}")
    scf = scr[:]
    n2t = sml.tile([128, 1024], F32R, tag="n2t", name=f"n2t_{rep}")

    # fdpj[cc][j]: [128, 34, 32] f32r; global padded-downsample col x at
    # local x-j; interior value fdp[y, x] = fp2[2y-1, 2x-1] (y,x in 1..32).
    fdpj = []
    for cc in range(2):
        row = []
        for j in range(3):
            t = fdj.tile([128, 34, 32], F32R, tag="fdj",
                         name=f"fdpj_{rep}_{cc}_{j}")
            nc.vector.tensor_copy(t[:, 0:1, :], zer[:, 0:32].unsqueeze(1))
            nc.vector.tensor_copy(t[:, 33:34, :], zer[:, 0:32].unsqueeze(1))
            x_lo, x_hi = max(1, j), min(32, j + 31)
            if j == 0:
                nc.vector.tensor_copy(t[:, 1:33, 0:1], zer[:, 0:32].unsqueeze(2))
            if j == 2:
                nc.vector.tensor_copy(t[:, 1:33, 31:32], zer[:, 0:32].unsqueeze(2))
            nc.vector.tensor_copy(
                t[:, 1:17, x_lo - j:x_hi - j + 1],
                fp2[cc][:, 1:33:2, 2 * x_lo - 2:2 * x_hi - 1:2])
            nc.vector.tensor_copy(
                t[:, 17:33, x_lo - j:x_hi - j + 1],
                fp2[cc][:, 33:65:2, 2 * x_lo - 2:2 * x_hi - 1:2])
            row.append(t)
        fdpj.append(row)

    # plane[cc][(a,b,tw)][i, l] = fp2[cc][2i+a, 2(tw+l)+b]  (bf16, GpSimd)
    planes = []
    for cc in range(2):
        d = {}
        for a in range(2):
            for b in range(2):
                for tw in range(2):
                    t = pln.tile([128, 34, 32], BF16, tag="pln",
                                 name=f"pl_{rep}_{cc}_{a}_{b}_{tw}")
                    eng = nc.gpsimd
                    c0 = 2 * tw + b - 1
                    if c0 < 0:
                        eng.memset(t[:, :, 0:1], 0.0)
                        eng.tensor_copy(
                            t[:, :, 1:32], fp2[cc][:, a:68:2, 1:62:2])
                    elif c0 + 62 > 63:
                        eng.memset(t[:, :, 31:32], 0.0)
                        eng.tensor_copy(
                            t[:, :, 0:31], fp2[cc][:, a:68:2, c0:c0 + 61:2])
                    else:
                        eng.tensor_copy(
                            t[:], fp2[cc][:, a:68:2, c0:c0 + 63:2])
                    d[(a, b, tw)] = t
        planes.append(d)



    # ---------------- rnorm path (scr cols 0:2176) ----------------
    sq1 = scf[:, 0:1088].rearrange("p (a b) -> p a b", a=34)
    nbx = scf[:, 1088:2176].rearrange("p (a b) -> p a b", a=34)
    sq2 = scf[:, 1088:2112]
    nc.vector.tensor_tensor(sq1[:], fdpj[0][1][:].bitcast(F32),
                            fdpj[0][1][:].bitcast(F32), OP.mult)
    nc.vector.tensor_tensor(sq2[:, 0:1024].rearrange("p (a b) -> p a b", a=32),
                            fdpj[1][1][:, 1:33, :].bitcast(F32),
                            fdpj[1][1][:, 1:33, :].bitcast(F32), OP.mult)
    nc.vector.tensor_add(sq1[:, 1:33, :], sq1[:, 1:33, :],
                         sq2[:, 0:1024].rearrange("p (a b) -> p a b", a=32))
    nc.vector.tensor_copy(nbx[:], sq1[:])
    nc.vector.tensor_add(nbx[:, :, 1:32], nbx[:, :, 1:32], sq1[:, :, 0:31])
    nc.vector.tensor_add(nbx[:, :, 0:31], nbx[:, :, 0:31], sq1[:, :, 1:32])
    n2 = n2t[:].rearrange("p (a b) -> p a b", a=32)
    nc.vector.tensor_add(n2[:], nbx[:, 0:32, :], nbx[:, 1:33, :])
    nc.vector.tensor_add(n2[:], n2[:], nbx[:, 2:34, :])
    nrm = scf[:, 0:1024]
    rnorm_q = sml.tile([128, 8], F32, tag="rnorm_q")

    def _emit_norm_matmuls():
        for hh in range(2):
            ps = psp.tile([128, 512], F32, tag="ps", name=f"nps_{rep}_{hh}")
            nc.tensor.matmul(ps[:], ones_r[:], n2t[:, 512 * hh:512 * hh + 512],
                             start=True, stop=True)
            nc.scalar.sqrt(nrm[:, 512 * hh:512 * hh + 512], ps[:])
        nc.vector.reciprocal(nrm[:], nrm[:])
        # scatter the 1024-wide row into [128, 8] via tiny PE outer products
        psn = psp.tile([128, 16], F32, tag="ps", name=f"rnq_{rep}")
        for c8 in range(8):
            nc.tensor.matmul(psn[:, 2 * c8:2 * c8 + 2],
                             nrm[0:1, 128 * c8:128 * (c8 + 1)],
                             onesf[0:1, 0:2], start=True, stop=True)
        nc.vector.tensor_copy(rnorm_q[:], psn[:, 0:16:2])

    # ---------------- Gram -> M0 (guarded [128, 8, DW]) ----------------
    M0 = big.tile([128, 8, DW], F32R, tag="big", name=f"m0_{rep}")
    zb = zer[:, 0:G].unsqueeze(1).to_broadcast([128, 8, G])
    nc.vector.tensor_copy(M0[:, :, 0:G], zb)
    nc.vector.tensor_copy(M0[:, :, G + 1024:], zb)
    shifts = [(i, j) for i in range(3) for j in range(3)]
    # The unnormalized gram U is symmetric: compute only p-blocks b >= b0(t)
    # (upper triangle, in >=256-wide groups for the f32r fast path) and fill
    # the lower triangle with PE transposes of unscaled row buffers.
    urow = big.tile([128, 896], F32R, tag="big", name=f"urow_{rep}")
    deferred = []   # scaled evacs parked until rnorm_q exists

    def _scaled_evac(t, c0, c1, ps, o):
        dst = M0[:, t, G + c0:G + c1]
        src = ps[:, o:o + c1 - c0]
        if t % 2 == 0:
            nc.scalar.mul(dst, src, rnorm_q[:, t:t + 1])
        else:
            nc.vector.tensor_scalar_mul(dst, src, rnorm_q[:, t:t + 1])

    for t in range(8):
        if t == 1:
            _emit_norm_matmuls()
            for args in deferred:
                _scaled_evac(*args)
            deferred = []
        b0 = 6 if t == 7 else t
        nb = 8 - b0
        groups = [(b0, min(nb, 4))]
        if nb > 4:
            groups.append((b0 + 4, nb - 4))
        rowps = []
        defer_t = t < 1

        def evac(*args):
            if defer_t:
                deferred.append(args)
            else:
                _scaled_evac(*args)
        for (gb, gn) in groups:
            ps = psp.tile([128, 512], F32, tag="ps", name=f"gps_{rep}_{t}_{gb}")
            k = 0
            for cc in range(2):
                for (i, j) in shifts:
                    lhsT = fdpj[cc][j][:, i + 4 * t:i + 4 * t + 4, :]
                    rhs = fdpj[cc][j][:, i + 4 * gb:i + 4 * gb + 4 * gn, :]
                    nc.tensor.matmul(ps[:, 0:128 * gn], lhsT, rhs,
                                     start=(k == 0), stop=(k == 17))
                    k += 1
            rowps.append((gb, gn, ps))
            evac(t, 128 * gb, 128 * (gb + gn), ps, 0)
        # unscaled evac of blocks b > t into urow (transpose sources)
        if t < 7:
            for (gb, gn, ps) in rowps:
                lo = max(gb, t + 1)
                if lo < gb + gn:
                    nc.vector.tensor_copy(
                        urow[:, 128 * (lo - t - 1):128 * (gb + gn - t - 1)],
                        ps[:, 128 * (lo - gb):128 * gn])
            # lower-triangle fills: dest (a, col-block t) for a > t
            dests = [a for a in range(t + 1, 8) if not (a == 7 and t == 6)]
            for a0 in range(0, len(dests), 4):
                grp = dests[a0:a0 + 4]
                pst = psp.tile([128, 512], F32R, tag="ps",
                               name=f"gtr_{rep}_{t}_{a0}")
                for k2, a in enumerate(grp):
                    nc.tensor.transpose(pst[:, 128 * k2:128 * (k2 + 1)],
                                        urow[:, 128 * (a - t - 1):128 * (a - t)],
                                        ident_r[:])
                for k2, a in enumerate(grp):
                    evac(a, 128 * t, 128 * (t + 1), pst, 128 * k2)

    if dbg_d is not None:
        nc.sync.dma_start(dbg_d["d_m0"][:], M0[:].bitcast(F32))
        for _cc in range(2):
            for _j in range(3):
                nc.sync.dma_start(dbg_d["d_fj"][:, 3 * _cc + _j],
                                  fdpj[_cc][_j][:].bitcast(F32))
        nc.sync.dma_start(dbg_d["d_rn"][:], rnorm_q[:])

    # ------------- mask -> mm_q [128, 8] (scr reused after norm) -------
    for k, (dy, dx) in enumerate(((0, 0), (0, 1), (1, 0), (1, 1))):
        off = 0 if k == 0 else 1024
        dst = scf[0:1, off:off + 1024].rearrange("o (a b) -> o a b", a=32)
        nc.sync.dma_start(dst, mask0[dy::4, dx::4][None])
        if k > 0:
            nc.gpsimd.tensor_tensor(scf[0:1, 0:1024], scf[0:1, 0:1024],
                                    scf[0:1, 1024:2048], OP.add)
    msum = scf[0:1, 0:1024].rearrange("o (a b) -> o a b", a=32)
    mdp = scf[0:1, 1024:2180].rearrange("o (a b) -> o a b", a=34)
    mbx = scf[0:1, 0:1088].rearrange("o (a b) -> o a b", a=34)
    nc.gpsimd.memset(mdp[:], 0.0)
    nc.gpsimd.tensor_scalar(mdp[:, 1:33, 1:33], msum[:], 2.5, None, OP.is_ge)
    nc.gpsimd.tensor_tensor(mbx[:], mdp[:, :, 0:32], mdp[:, :, 1:33], OP.add)
    nc.gpsimd.tensor_tensor(mbx[:], mbx[:], mdp[:, :, 2:34], OP.add)
    mbox = scf[0:1, 1088:2112].rearrange("o (a b) -> o a b", a=32)
    nc.gpsimd.tensor_tensor(mbox[:], mbx[:, 0:32, :], mbx[:, 1:33, :], OP.add)
    nc.gpsimd.tensor_tensor(mbox[:], mbox[:], mbx[:, 2:34, :], OP.add)
    mmrow = scf[0:1, 0:1024]
    nc.gpsimd.tensor_scalar(mmrow[:].rearrange("o (a b) -> o a b", a=32),
                            mbox[:], 0.0, None, OP.is_equal)
    mm_q = sml.tile([128, 8], F32, tag="mm_q")
    for c8 in range(8):
        nc.sync.dma_start(mm_q[:, c8:c8 + 1], mmrow[:, 128 * c8:128 * (c8 + 1)])

    if dbg_d is not None:
        nc.sync.dma_start(dbg_d["d_mm"][:], mm_q[:])

    # RW[cc]: [128 l, 8 qc, 16 tap, 2 (hi,lo), 128 c] fp8 block layout
    # (DoubleRow weights need 16B-aligned slot strides). Per tap the 8
    # bf16 transposes fill one full PSUM bank; hi = e4m3 round (ScalarE
    # copy), lo = e4m3(x - hi) (DVE subtract) evacuate it. rw0's build is
    # interleaved with the fuse phase (PSUM/engines have slack); rw1 is
    # built after the softmax so its evacs overlap deconv-cc0's PE time.
    def _rw_direct(rw, cc, u, v):
        a, s = u % 2, u // 2
        b, tw = v % 2, v // 2
        pl_t = planes[cc][(a, b, tw)]
        ps = psp.tile([128, 1024], BF16, tag="ps",
                      name=f"rwt_{rep}_{cc}_{u}_{v}")
        for qc in range(8):
            nc.tensor.transpose(
                ps[:, 128 * qc:128 * (qc + 1)],
                pl_t[:, s + 4 * qc:s + 4 * qc + 4, :],
                ident_bf[:])
        src = ps[:].rearrange("p (k c) -> p k c", k=8)
        hid = rw[:, :, 4 * u + v, 0, :]
        lod = rw[:, :, 4 * u + v, 1, :]
        nc.scalar.copy(hid, src)
        nc.vector.tensor_tensor(lod, src, hid, OP.subtract)

    # taps (u+2, v) are 32-partition shifts of taps (u, v): derive the
    # bulk with two partition-shifted SBUF DMAs (idle DMA device); only
    # the last coarse row (l_y=31, partitions 96..127 of qc=7) needs a
    # fresh one-strip transpose + hi/lo evac.
    def _rw_derived(rw, cc, u, v):
        st = 4 * (u - 2) + v
        dt = 4 * u + v
        nc.sync.dma_start(rw[0:96, :, dt], rw[32:128, :, st])
        nc.sync.dma_start(rw[96:128, 0:7, dt], rw[0:32, 1:8, st])
        a, s = u % 2, u // 2
        b, tw = v % 2, v // 2
        pl_t = planes[cc][(a, b, tw)]
        ps = psp.tile([128, 128], BF16, tag="ps",
                      name=f"rws_{rep}_{cc}_{u}_{v}")
        nc.tensor.transpose(ps[:], pl_t[:, s + 28:s + 32, :],
                            ident_bf[:])
        hid = rw[96:128, 7, dt, 0, :]
        lod = rw[96:128, 7, dt, 1, :]
        nc.scalar.copy(hid, ps[96:128, :])
        nc.vector.tensor_tensor(lod, ps[96:128, :], hid, OP.subtract)

    rw0 = big.tile([128, 8, 16, 2, 128], F8, tag="big", name=f"rw_{rep}_0")
    RW = [rw0, None]

    # ---------------- fuse1 (diag +-1, row-major) ----------------
    M1 = big.tile([128, 8, DW], F32R, tag="big", name=f"m1_{rep}")
    nc.vector.tensor_copy(M1[:, :, 0:G], zb)
    nc.vector.tensor_copy(M1[:, :, G + 1024:], zb)
    for ch in range(8):
        for hh in range(2):
            ps = psp.tile([128, 512], F32, tag="ps", name=f"f1_{rep}_{ch}_{hh}")
            src = slice(G + 1 + 512 * hh, G + 513 + 512 * hh)
            srcm = slice(G - 1 + 512 * hh, G + 511 + 512 * hh)
            terms = [(ps[:], sh["p1"][:], M0[:, ch, src])]
            if ch < 7:
                terms.append((ps[:], sh["bp1"][:], M0[:, ch + 1, src]))
            terms.append((ps[:], sh["m1"][:], M0[:, ch, srcm]))
            if ch > 0:
                terms.append((ps[:], sh["bm1"][:], M0[:, ch - 1, srcm]))
            _emit_group(nc, terms)
            nc.vector.tensor_add(M1[:, ch, G + 512 * hh:G + 512 * hh + 512],
                                 M0[:, ch, G + 512 * hh:G + 512 * hh + 512],
                                 ps[:])
        _rw_direct(rw0, 0, ch // 4, ch % 4)

    if dbg_d is not None:
        nc.sync.dma_start(dbg_d["d_m1"][:], M1[:].bitcast(F32))

    # ---------------- fuse2 (diag +-1, col-major w/ wrap) ----------------
    M2 = big.tile([128, 8, DW], F32R, tag="big", name=f"m2_{rep}")
    nc.vector.tensor_copy(M2[:, :, 0:G], zb)
    nc.vector.tensor_copy(M2[:, :, G + 1024:], zb)
    for ch in range(8):
        up = sh["bp32"] if ch < 7 else sh["wp"]
        up_src = ch + 1 if ch < 7 else 0
        dn = sh["bm32"] if ch > 0 else sh["wm"]
        dn_src = ch - 1 if ch > 0 else 7
        nc.vector.tensor_copy(slv["pm"][:, 0:31], M1[:, ch, G + 1:G + 32])
        nc.vector.tensor_copy(slv["pb"][:, 0:31], M1[:, up_src, G + 1:G + 32])
        nc.vector.tensor_copy(slv["mm"][:, 1:32], M1[:, ch, G + 992:G + 1023])
        nc.vector.tensor_copy(slv["mb"][:, 1:32], M1[:, dn_src, G + 992:G + 1023])
        for hh in range(2):
            ps = psp.tile([128, 512], F32, tag="ps", name=f"f2_{rep}_{ch}_{hh}")
            if hh == 0:
                sp = slice(G + 32, G + 544)
                terms = [
                    (ps[:], sh["p32"][:], M1[:, ch, sp]),
                    (ps[:], up[:], M1[:, up_src, sp]),
                    (ps[:, 32:512], sh["m32"][:], M1[:, ch, G:G + 480]),
                    (ps[:, 32:512], dn[:], M1[:, dn_src, G:G + 480]),
                    (ps[:, 0:32], sh["m32"][:], slv["mm"][:]),
                    (ps[:, 0:32], dn[:], slv["mb"][:]),
                ]
            else:
                sm = slice(G + 480, G + 992)
                terms = [
                    (ps[:], sh["m32"][:], M1[:, ch, sm]),
                    (ps[:], dn[:], M1[:, dn_src, sm]),
                    (ps[:, 0:480], sh["p32"][:], M1[:, ch, G + 544:G + 1024]),
                    (ps[:, 0:480], up[:], M1[:, up_src, G + 544:G + 1024]),
                    (ps[:, 480:512], sh["p32"][:], slv["pm"][:]),
                    (ps[:, 480:512], up[:], slv["pb"][:]),
                ]
            _emit_group(nc, terms)
            nc.vector.tensor_add(M2[:, ch, G + 512 * hh:G + 512 * hh + 512],
                                 M1[:, ch, G + 512 * hh:G + 512 * hh + 512],
                                 ps[:])
        nc.scalar.mul(M2[:, ch, G:G + 1024], M2[:, ch, G:G + 1024],
                      mm_q[:, ch:ch + 1])
        _rw_derived(rw0, 0, 2 + ch // 4, ch % 4)

    if dbg_d is not None:
        nc.sync.dma_start(dbg_d["d_m2"][:], M2[:].bitcast(F32))

    # ---------------- RW deconv banks already built (see fuse/softmax) --

    # ---------------- p-major softmax -> Ax ----------------
    # A stored ONCE: [128 l, 273 rows, 32 cols] u16 cells, fp8 value in the
    # low byte, 0 in the high byte. Rows = 1 global pad + 8 qc-blocks of 34
    # (pad, 32 data, pad). Every deconv window (any sx shift) is then a
    # stride-2 fp8 run starting at an EVEN byte -- a legal DoubleRow moving
    # AP. Row-boundary wrap artifacts of the +-1 col shifts are cancelled
    # by small correction matmuls against negated edge columns (Axe).
    Ax16 = fpa.tile([128, 274, 32], U16, tag="fpa", name=f"ap_{rep}")
    nc.gpsimd.memset(Ax16[:, 0:137, :], 0.0)
    nc.gpsimd.memset(Ax16[:, 137:274, :], 0.0)
    Axv = Ax16[:].bitcast(F8)                       # [128, 274, 64]
    Axq = Axv[:, 1:273, :].rearrange(
        "p (q r) c -> p q r c", q=8)                # [128, 8, 34, 64]
    Axe = sml.tile([128, 2, 274], F8, tag="axe", name=f"axe_{rep}")
    mx8 = sml.tile([128, 8, 2], F32, tag="mx8")
    bias8 = sml.tile([128, 8], F32, tag="bias8")
    den8 = sml.tile([128, 8, 2], F32, tag="den8")
    rcp8 = sml.tile([128, 8], F32, tag="rcp8")
    diag = sml.tile([128, 4, 128], BF16, tag="diag")
    nc.scalar.activation(zf[0:1, 64:66], zf[0:1, 64:66], ACT.Exp)  # preload
    LAG = 2  # transpose-back trails the max/exp chain by LAG p-blocks
    Ets = {}

    def _sm_front(pt):
        Et = etp.tile([128, 1024], BF16, tag="et", name=f"et_{rep}_{pt}")
        Ets[pt] = Et
        pss = []
        for gq in range(2):
            ps = psp.tile([128, 512], F32R, tag="ps", name=f"mt_{rep}_{pt}_{gq}")
            for k in range(4):
                t = 4 * gq + k
                nc.tensor.transpose(ps[:, 128 * k:128 * (k + 1)],
                                    M2[:, t, G + 128 * pt:G + 128 * pt + 128],
                                    ident_r[:])
            nc.vector.reduce_max(mx8[:, pt, gq:gq + 1], ps[:].bitcast(F32),
                                 axis=AX.X)
            pss.append(ps)
        nc.vector.tensor_tensor(bias8[:, pt:pt + 1], mx8[:, pt, 0:1],
                                mx8[:, pt, 1:2], OP.max)
        nc.vector.tensor_scalar_mul(bias8[:, pt:pt + 1], bias8[:, pt:pt + 1],
                                    -SCALE)
        for gq in range(2):
            nc.scalar.activation(
                Et[:, 512 * gq:512 * gq + 512], pss[gq][:].bitcast(F32),
                ACT.Exp, bias=bias8[:, pt:pt + 1], scale=SCALE,
                accum_out=den8[:, pt, gq:gq + 1])
        nc.vector.tensor_add(rcp8[:, pt:pt + 1], den8[:, pt, 0:1],
                             den8[:, pt, 1:2])
        nc.vector.reciprocal(rcp8[:, pt:pt + 1], rcp8[:, pt:pt + 1])
        nc.vector.tensor_scalar_mul(diag[:, pt % 4, :], ident_bf[:],
                                    rcp8[:, pt:pt + 1])

    def _sm_back(pt):
        Et = Ets.pop(pt)
        for tg in range(2):
            ps = psp.tile([128, 512], F32, tag="ps", name=f"eq_{rep}_{pt}_{tg}")
            for k in range(4):
                t = 4 * tg + k
                nc.tensor.matmul(ps[:, 128 * k:128 * (k + 1)],
                                 Et[:, 128 * t:128 * t + 128],
                                 diag[:, pt % 4, :],
                                 start=True, stop=True)
            # no post-softmax mask multiply: masked-q rows carry weights
            # <= e^-30 (fp8 flushes them to exact 0), so one batched
            # stride-2 copy into the u16 cells per PSUM tile suffices.
            src = ps[:].rearrange("p (k a b) -> p k a b", k=4, a=4)
            rows = slice(1 + 4 * pt, 5 + 4 * pt)
            ts = slice(4 * tg, 4 * tg + 4)
            dst = Axq[:, ts, rows, 0:64:2]
            if (pt + tg) % 2 == 0:
                nc.vector.tensor_copy(dst, src)
            else:
                nc.scalar.copy(dst, src)

    for pt in range(8 + LAG):
        if pt < 8:
            _sm_front(pt)
        if pt >= LAG:
            _sm_back(pt - LAG)
    # negated edge columns for the deconv wrap corrections
    nc.vector.tensor_scalar(Axe[:, 0, :], Axv[:, :, 0], -1.0, None, OP.mult)
    nc.vector.tensor_scalar(Axe[:, 1, :], Axv[:, :, 62], -1.0, None, OP.mult)

    # rw1 build lands here: its hi/lo evacs run on DVE/ScalarE while the
    # deconv for cc=0 keeps the PE busy.
    rw1 = big.tile([128, 8, 16, 2, 128], F8, tag="big", name=f"rw_{rep}_1")
    RW[1] = rw1
    for u in range(2):
        for v in range(4):
            _rw_direct(rw1, 1, u, v)
    for u in range(2, 4):
        for v in range(4):
            _rw_derived(rw1, 1, u, v)

    if dbg_d is not None:
        nc.sync.dma_start(dbg_d["d_ap"][:], Axa[:])
        for _pt in range(8):
            pass

    # ---------------- deconv ----------------
    Axf = Axv.rearrange("p r c -> p (r c)")          # [128, 17536]
    corr_sb = sml.tile([128, 64], F32, tag="corrsb")
    for cc in range(2):
        out_sb = big.tile([128, 64, 64], F32, tag="big", name=f"os_{rep}_{cc}")
        for hh in range(2):
            # each acc completes serially: its evac (alternating ScalarE/DVE)
            # overlaps the next acc's matmuls, shrinking the store tail.
            # Main matmuls read flat stride-2 fp8 windows of the u16 cells;
            # the +-1 col shifts wrap at row edges; N=16 correction matmuls
            # vs the negated edge cols (Axe) accumulate into corr_ps, added
            # back onto the 16 edge pixels after the evac.
            corr_ps = psp.tile([128, 64], F32, tag="ps",
                               name=f"dc_{rep}_{cc}_{hh}")
            for ry in range(2):
                us = [u for u in range(4) if (u + 1) % 2 == ry]
                for rx in range(2):
                    vs = [v for v in range(4) if (v + 1) % 2 == rx]
                    acc = psp.tile([128, 512], F32, tag="ps",
                                   name=f"da_{rep}_{cc}_{hh}_{ry}_{rx}")
                    k = 0
                    fixes = []
                    for qc in range(8):
                        for u in us:
                            for v in vs:
                                sy = (ry + 1 - u) // 2
                                sx = (rx + 1 - v) // 2
                                rlo = 1 + sy + 16 * hh
                                r0 = 1 + 34 * qc + rlo
                                st2 = 64 * r0 + 2 * sx
                                win = Axf[:, st2:st2 + 1024:2]
                                rhs = win.unsqueeze(1).to_broadcast(
                                    [128, 2, 512])
                                if sx != 0:
                                    fixes.append((qc, u, v, sx, r0))
                                nc.tensor.matmul(acc[:],
                                                 RW[cc][:, qc, 4 * u + v],
                                                 rhs, start=(k == 0),
                                                 stop=(k == 31),
                                                 perf_mode=DROW)
                                k += 1
                    j = 2 * ry + rx
                    nf = len(fixes)
                    for fi, (qc, u, v, sx, r0) in enumerate(fixes):
                        if sx == 1:
                            rhsE = Axe[:, 0, r0 + 1:r0 + 17]
                        else:
                            rhsE = Axe[:, 1, r0 - 1:r0 + 15]
                        rhs = rhsE.unsqueeze(1).to_broadcast([128, 2, 16])
                        nc.tensor.matmul(corr_ps[:, 16 * j:16 * j + 16],
                                         RW[cc][:, qc, 4 * u + v],
                                         rhs, start=(fi == 0),
                                         stop=(fi == nf - 1),
                                         perf_mode=DROW,
                                         skip_group_check=True)
                    dst = out_sb[:, 32 * hh + ry:32 * (hh + 1):2, rx::2]
                    if (ry + rx) % 2 == 0:
                        nc.scalar.mul(dst, acc[:], 0.25)
                    else:
                        nc.vector.tensor_scalar(dst, acc[:], 0.25, None,
                                                OP.mult)
            nc.vector.tensor_scalar(corr_sb[:], corr_ps[:], 0.25, None,
                                    OP.mult)
            for ry in range(2):
                for rx in range(2):
                    j = 2 * ry + rx
                    ecol = 63 if rx == 1 else 0
                    edge = out_sb[:, 32 * hh + ry:32 * (hh + 1):2,
                                  ecol:ecol + 1].squeeze(2)
                    nc.vector.tensor_add(edge, edge,
                                         corr_sb[:, 16 * j:16 * j + 16])
            for qh in range(2):
                r0 = 32 * hh + 16 * qh
                nc.sync.dma_start(out_d[cc * 128:(cc + 1) * 128, r0:r0 + 16, :],
                                  out_sb[:, r0:r0 + 16, :])


_NC_CACHE = {}


def _get_nc(cfg=("f32r", "bf16")):
    if cfg not in _NC_CACHE:
        _NC_CACHE[cfg] = build_nc(*cfg)
    return _NC_CACHE[cfg]


def kernel(feature: np.ndarray, mask: np.ndarray) -> np.ndarray:
    feature = np.ascontiguousarray(np.asarray(feature, dtype=np.float32))
    mask = np.asarray(mask, dtype=np.float32)
    nc = _get_nc()
    m0 = np.ascontiguousarray(mask[0, 0])
    in_maps = [{"feature": np.ascontiguousarray(feature[i]), "mask0": m0}
               for i in range(N_CORES)]
    res = run_bass_kernel_spmd(nc, in_maps, list(range(N_CORES)))
    return np.stack([np.asarray(res.results[i]["out"], dtype=np.float32)
                     for i in range(N_CORES)])

